# revision 1
# baseline (speedup 1.0000x reference)
"""Trainium2 Bass kernel for ragged segment-max + 1x1 conv + GeM pooling.

Problem: x [1,128,4096,16,11] f32 packed frames; seqL [32] ragged lengths;
W [256,128] 1x1-conv weight; p [4] GeM powers.  out [32, 256, 4] f32.

Strategy: shard whole segments across 8 cores (4 per core, LPT-balanced).
Per core: stream frames through a DVE max-reduce at 16-frame chunk
granularity (chunks segment-aligned via -1e30 padding; chunk data
transposed hw-major on host so the reduce inner dim is contiguous), then a
segmented max-scan over chunk maxes combines them into per-segment maxes
(reset gates at segment starts, uploaded as data so the program is uniform
across cores).  Segment results are gathered at data-driven chunk indices
via register-offset copies, pushed through the 1x1 conv on the PE, and the
GeM tail (clip, ln, *p, exp, mean, ^(1/p)) runs on ACT/DVE.
"""
import sys

import numpy as np

if "/opt/trn_rl_repo" not in sys.path:
    sys.path.insert(0, "/opt/trn_rl_repo")

# problem constants
B, S, C, O = 32, 4096, 128, 256
H, Wd = 16, 11
HW = H * Wd                  # 176
SPLIT = [4, 4, 4, 4]         # h split sizes
NPART = len(SPLIT)           # 4
WPP = HW // NPART            # 44 elems per GeM part
EPS = 1e-6
NCORES = 8
SEG_PER_CORE = B // NCORES   # 4

R = 16                       # frames per chunk (segment-alignment quantum)
CPB = 3                      # chunks per DMA buffer
BIG = 3.0e38
PAD = -1.0e30
RESCALE = 150.0              # GeM computed on t*RESCALE: ACT Ln table is only
                             # valid on ~[2^-64, 2^64], so keep (RESCALE*t)^p
                             # inside it for t in [EPS, ~50]
SMIN = 44.0 * 2.0 ** -60     # clamp sum(w2) so mean stays in the Ln window


_prog_cache = {}
DEBUG = False


def _plan(seqL):
    """Host planning: segment->core assignment + per-core chunk layout."""
    seqL = np.asarray(seqL).astype(np.int64).reshape(B)
    assert seqL.sum() == S and (seqL > 0).all()
    starts = np.concatenate([[0], np.cumsum(seqL)[:-1]])
    chunks = (seqL + R - 1) // R  # padded chunk count per segment

    # LPT: assign segments to cores balancing padded chunk totals, 4 per core
    order = np.argsort(-chunks, kind="stable")
    loads = [0] * NCORES
    members = [[] for _ in range(NCORES)]
    for sid in order:
        cand = sorted(range(NCORES), key=lambda c: (loads[c], c))
        for c in cand:
            if len(members[c]) < SEG_PER_CORE:
                members[c].append(int(sid))
                loads[c] += int(chunks[sid])
                break
    for c in range(NCORES):
        members[c].sort()

    ncch = [sum(int(chunks[s]) for s in members[c]) for c in range(NCORES)]
    NCH = max(ncch)
    NCH = ((NCH + CPB - 1) // CPB) * CPB  # multiple of chunks-per-buffer
    return {
        "seqL": seqL, "starts": starts, "chunks": chunks,
        "members": members, "NCH": NCH, "NB": NCH // CPB,
    }


def _repack_core(x_chw, plan, core):
    """Per-core DRAM stream [C, NCH*HW*R] (chunk-major; each chunk stored
    hw-major [hw, frame]), plus scan gate row and segment-end chunk ids."""
    NCH = plan["NCH"]
    members = plan["members"][core]
    out = np.full((C, NCH, HW, R), PAD, dtype=np.float32)
    gate_row = np.full((NCH,), BIG, dtype=np.float32)
    ends = np.zeros((SEG_PER_CORE,), dtype=np.int32)
    cpos = 0
    for j, sid in enumerate(members):
        L = int(plan["seqL"][sid]); s0 = int(plan["starts"][sid])
        k = int(plan["chunks"][sid])
        segp = np.full((C, k * R, HW), PAD, dtype=np.float32)
        segp[:, :L, :] = x_chw[:, s0:s0 + L, :]
        out[:, cpos:cpos + k] = segp.reshape(C, k, R, HW).transpose(0, 1, 3, 2)
        gate_row[cpos] = -BIG                           # reset at segment start
        cpos += k
        ends[j] = cpos - 1                              # last chunk of segment
    return out.reshape(C, NCH * HW * R), gate_row, ends


def _build_program(NCH, NB):
    import concourse.bass as bass
    import concourse.tile as tile
    from concourse import bacc, mybir

    F32 = mybir.dt.float32
    FREE_B = CPB * R * HW  # elems per partition per DMA buffer
    SH = SEG_PER_CORE * HW          # 704
    SN = SEG_PER_CORE * NPART       # 16

    nc = bacc.Bacc("TRN2", target_bir_lowering=False, debug=False,
                   num_devices=NCORES)
    x = nc.dram_tensor("x", [C, NCH * HW * R], F32, kind="ExternalInput")
    gate_d = nc.dram_tensor("gate", [C, HW * NCH], F32, kind="ExternalInput")
    gidx_d = nc.dram_tensor("gidx", [C, SEG_PER_CORE], mybir.dt.int32,
                            kind="ExternalInput")
    wt_d = nc.dram_tensor("wt", [C, O], F32, kind="ExternalInput")
    pvec_d = nc.dram_tensor("pvec", [C, 2 * SH], F32, kind="ExternalInput")
    qvec_d = nc.dram_tensor("qvec", [C, 2 * SN], F32, kind="ExternalInput")
    out_d = nc.dram_tensor("out", [C, 2 * SN], F32, kind="ExternalOutput")
    if DEBUG:
        dbg_d = nc.dram_tensor("dbg", [C, 5 * SH + 3 * SN], F32,
                               kind="ExternalOutput")

    with tile.TileContext(nc) as tc:
        with tc.tile_pool(name="xin", bufs=3) as xin, \
             tc.tile_pool(name="meta", bufs=1) as meta, \
             tc.tile_pool(name="work", bufs=1) as work, \
             tc.tile_pool(name="dram", bufs=1, space="DRAM") as dramp, \
             tc.tile_pool(name="psum", bufs=1, space="PSUM") as psum:
            cmax = work.tile([C, HW * NCH], F32, tag="cmax")
            gate = meta.tile([C, HW * NCH], F32, tag="gate")
            wt = meta.tile([C, O], F32, tag="wt")
            pvec = meta.tile([C, 2 * SH], F32, tag="pvec")
            qvec = meta.tile([C, 2 * SN], F32, tag="qvec")
            gidx = meta.tile([C, SEG_PER_CORE], mybir.dt.int32, tag="gidx")

            # warm the ACT Ln/Exp tables during streaming (their lazy
            # loads otherwise land in the serial tail)
            warm = work.tile([C, 1], F32, tag="warm")
            nc.vector.memset(warm[:], 1.0)
            nc.scalar.activation(warm[:], warm[:],
                                 mybir.ActivationFunctionType.Ln)
            nc.scalar.activation(warm[:], warm[:],
                                 mybir.ActivationFunctionType.Exp)

            # phase 1: stream buffers, per-chunk max-reduce
            for b in range(NB):
                t = xin.tile([C, FREE_B], F32, tag="xin")
                nc.sync.dma_start(t[:], x[:, b * FREE_B:(b + 1) * FREE_B])
                src = t[:].rearrange("p (c h r) -> p c h r", c=CPB, h=HW, r=R)
                dst = cmax[:].rearrange("p (h n) -> p h n", n=NCH)[
                    :, :, b * CPB:(b + 1) * CPB].rearrange("p h c -> p c h")
                nc.vector.reduce_max(dst, src, axis=mybir.AxisListType.X)

            # meta loads issued after the stream DMAs so they don't delay
            # the first x buffer on the HWDGE queues; gate goes via SWDGE
            nc.gpsimd.dma_start(gate[:], gate_d[:])
            nc.sync.dma_start(wt[:], wt_d[:])
            nc.sync.dma_start(pvec[:], pvec_d[:])
            nc.sync.dma_start(qvec[:], qvec_d[:])
            nc.sync.dma_start(gidx[:], gidx_d[:])

            # phase 2: segmented max-scan along chunk axis (resets via gate).
            # Split by h-halves so the ACT transpose of half 0 overlaps the
            # DVE scan of half 1 (scans are independent per hw position).
            sout = xin.tile([C, HW * NCH], F32, tag="xin")
            HHALF = (HW // 2) * NCH
            for hh in range(2):
                sl = slice(hh * HHALF, (hh + 1) * HHALF)
                nc.vector.tensor_tensor_scan(
                    sout[:, sl], gate[:, sl], cmax[:, sl], initial=-BIG,
                    op0=mybir.AluOpType.min, op1=mybir.AluOpType.max)

            # transpose scan output to chunk-major on ACT (idle engine),
            # round-trip through DRAM, then gather per-segment slices as an
            # indirect row-gather (row p*NCH + end_chunk of the [C*NCH, HW]
            # view; indices are host data so the program is core-uniform)
            soutT = xin.tile([C, HW * NCH], F32, tag="xin")
            for hh in range(2):
                h0 = hh * (HW // 2)
                nc.scalar.copy(
                    soutT[:].rearrange("p (n h) -> p n h", h=HW)[
                        :, :, h0:h0 + HW // 2],
                    sout[:].rearrange("p (h n) -> p h n", n=NCH)[
                        :, h0:h0 + HW // 2, :].rearrange("p h n -> p n h"))
            scratch = dramp.tile([C, NCH * HW], F32, tag="scratch")
            nc.sync.dma_start(scratch[:], soutT[:])
            table = scratch[:].rearrange("p (n h) -> (p n) h", h=HW)
            pooled = work.tile([C, SH], F32, tag="pooled")
            for j in range(SEG_PER_CORE):
                nc.gpsimd.indirect_dma_start(
                    out=pooled[:, j * HW:(j + 1) * HW],
                    out_offset=None,
                    in_=table,
                    in_offset=bass.IndirectOffsetOnAxis(
                        ap=gidx[:, j:j + 1], axis=0))

            # conv 1x1 (2 O-halves x 2 psum banks each) + GeM tail.
            # Both halves share each op so the ACT engine loads each
            # activation table once instead of reloading per switch.
            gtile = work.tile([C, 2 * SN], F32, tag="g")
            t1 = work.tile([C, 2 * SH], F32, tag="t1x")
            for half in range(2):
                for ns in range(2):
                    y = psum.tile([C, SH // 2], F32, tag=f"y{half}{ns}")
                    nc.tensor.matmul(
                        y[:],
                        wt[:, half * 128:(half + 1) * 128],
                        pooled[:, ns * (SH // 2):(ns + 1) * (SH // 2)],
                        start=True, stop=True)
                    nc.vector.tensor_scalar_max(
                        t1[:, half * SH + ns * (SH // 2):
                           half * SH + (ns + 1) * (SH // 2)], y[:], EPS)
            u = work.tile([C, 2 * SH], F32, tag="ux")
            nc.scalar.activation(u[:], t1[:],
                                 mybir.ActivationFunctionType.Ln,
                                 scale=float(RESCALE))
            v = work.tile([C, 2 * SH], F32, tag="vx")
            nc.vector.tensor_mul(v[:], u[:], pvec[:])
            w2 = work.tile([C, 2 * SH], F32, tag="wx")
            nc.scalar.activation(w2[:], v[:],
                                 mybir.ActivationFunctionType.Exp)
            s = work.tile([C, 2 * SN], F32, tag="sx")
            nc.vector.reduce_sum(
                s[:].rearrange("p (k one) -> p k one", one=1),
                w2[:].rearrange("p (k m) -> p k m", m=WPP),
                axis=mybir.AxisListType.X)
            nc.vector.tensor_scalar_max(s[:], s[:], float(SMIN))
            r2 = work.tile([C, 2 * SN], F32, tag="rx")
            nc.scalar.activation(r2[:], s[:],
                                 mybir.ActivationFunctionType.Ln,
                                 scale=float(1.0 / WPP))
            q2 = work.tile([C, 2 * SN], F32, tag="qx")
            nc.vector.tensor_mul(q2[:], r2[:], qvec[:])
            nc.vector.tensor_scalar_sub(q2[:], q2[:],
                                        float(np.log(RESCALE)))
            nc.scalar.activation(gtile[:], q2[:],
                                 mybir.ActivationFunctionType.Exp)
            nc.sync.dma_start(out_d[:], gtile[:])
            if DEBUG:
                nc.sync.dma_start(dbg_d[:, 0:SH], pooled[:])
                nc.sync.dma_start(dbg_d[:, SH:2 * SH], t1[:])
                nc.sync.dma_start(dbg_d[:, 2 * SH:3 * SH], u[:])
                nc.sync.dma_start(dbg_d[:, 3 * SH:4 * SH], v[:])
                nc.sync.dma_start(dbg_d[:, 4 * SH:5 * SH], w2[:])
                nc.sync.dma_start(dbg_d[:, 5 * SH:5 * SH + SN], s[:])
                nc.sync.dma_start(dbg_d[:, 5 * SH + SN:5 * SH + 2 * SN], r2[:])
                nc.sync.dma_start(dbg_d[:, 5 * SH + 2 * SN:], q2[:])
    nc.compile()
    return nc


def _run_device(nc, in_maps):
    from concourse.bass_utils import run_bass_kernel_spmd
    res = run_bass_kernel_spmd(nc, in_maps, list(range(NCORES)))
    return res.results


def _make_in_maps(x, plan, W, p):
    x_chw = np.ascontiguousarray(x[0]).reshape(C, S, HW)
    NCH = plan["NCH"]
    wt = np.ascontiguousarray(W.T).astype(np.float32)               # [C, O]
    prow = np.repeat(p.astype(np.float32), WPP)                     # [HW]
    pvec = np.ascontiguousarray(
        np.broadcast_to(np.tile(prow, 2 * SEG_PER_CORE)[None, :],
                        (C, 2 * SEG_PER_CORE * HW))).astype(np.float32)
    qrow = (1.0 / p.astype(np.float32))                             # [NPART]
    qvec = np.ascontiguousarray(
        np.broadcast_to(np.tile(qrow, 2 * SEG_PER_CORE)[None, :],
                        (C, 2 * SEG_PER_CORE * NPART))).astype(np.float32)
    in_maps = []
    for core in range(NCORES):
        xc, gate_row, ends = _repack_core(x_chw, plan, core)
        gate = np.ascontiguousarray(
            np.broadcast_to(
                np.repeat(gate_row[None, :], HW, axis=0).reshape(-1)[None, :],
                (C, HW * NCH))).astype(np.float32)
        in_maps.append({
            "x": xc, "gate": gate,
            "gidx": np.ascontiguousarray(
                (np.arange(C, dtype=np.int32)[:, None] * np.int32(NCH))
                + ends[None, :].astype(np.int32)),
            "wt": wt, "pvec": pvec, "qvec": qvec,
        })
    return in_maps


def kernel(x, seqL, W, p):
    x = np.asarray(x, dtype=np.float32)
    W = np.asarray(W, dtype=np.float32)
    p = np.asarray(p, dtype=np.float32)
    plan = _plan(seqL)

    in_maps = _make_in_maps(x, plan, W, p)

    key = (plan["NCH"], plan["NB"])
    if key not in _prog_cache:
        _prog_cache[key] = _build_program(plan["NCH"], plan["NB"])
    nc = _prog_cache[key]

    results = _run_device(nc, in_maps)

    SN = SEG_PER_CORE * NPART
    out = np.zeros((B, O, NPART), dtype=np.float32)
    for core in range(NCORES):
        g = results[core]["out"]  # [C, 2*SN]
        for j, sid in enumerate(plan["members"][core]):
            for half in range(2):
                blk = g[:, half * SN + j * NPART: half * SN + (j + 1) * NPART]
                out[sid, half * 128:(half + 1) * 128, :] = blk
    return out



# revision 6
# speedup vs baseline: 1.2140x; 1.2140x over previous
"""Trainium2 Bass kernel for ragged segment-max + 1x1 conv + GeM pooling.

Problem: x [1,128,4096,16,11] f32 packed frames; seqL [32] ragged lengths;
W [256,128] 1x1-conv weight; p [4] GeM powers.  out [32, 256, 4] f32.

Strategy: shard whole segments across 8 cores (4 per core, LPT-balanced).
Per core: stream frames through a DVE max-reduce at 16-frame chunk
granularity (chunks segment-aligned via -1e30 padding; chunk data
transposed hw-major on host so the reduce inner dim is contiguous).
Chunk maxes are folded straight into a 4-slot segment accumulator with
masked min/max ops on the (otherwise idle) GPSIMD engine: for each chunk,
acc[s] = max(acc[s], min(chunkmax, mask[chunk,s])) where mask is +/-BIG
host data - so the program stays uniform across cores while the
chunk->segment mapping is data.  The accumulator IS the pooled tensor:
the 1x1 conv on the PE and the GeM tail (clip, ln, *p, exp, mean,
^(1/p)) on ACT/DVE follow directly, with no scan / transpose / DRAM
round-trip / indirect gather.
"""
import sys

import numpy as np

if "/opt/trn_rl_repo" not in sys.path:
    sys.path.insert(0, "/opt/trn_rl_repo")

# problem constants
B, S, C, O = 32, 4096, 128, 256
H, Wd = 16, 11
HW = H * Wd                  # 176
SPLIT = [4, 4, 4, 4]         # h split sizes
NPART = len(SPLIT)           # 4
WPP = HW // NPART            # 44 elems per GeM part
EPS = 1e-6
NCORES = 8
SEG_PER_CORE = B // NCORES   # 4

R = 16                       # frames per chunk (segment-alignment quantum)
CPB = 3                      # chunks per DMA buffer
BIG = 3.0e38
PAD = -1.0e30
RESCALE = 150.0              # GeM computed on t*RESCALE: ACT Ln table is only
                             # valid on ~[2^-64, 2^64], so keep (RESCALE*t)^p
                             # inside it for t in [EPS, ~50]
SMIN = 44.0 * 2.0 ** -60     # clamp sum(w2) so mean stays in the Ln window


_prog_cache = {}


def _plan(seqL):
    """Host planning: segment->core assignment + per-core chunk layout."""
    seqL = np.asarray(seqL).astype(np.int64).reshape(B)
    assert seqL.sum() == S and (seqL > 0).all()
    starts = np.concatenate([[0], np.cumsum(seqL)[:-1]])
    chunks = (seqL + R - 1) // R  # padded chunk count per segment

    # LPT: assign segments to cores balancing padded chunk totals, 4 per core
    order = np.argsort(-chunks, kind="stable")
    loads = [0] * NCORES
    members = [[] for _ in range(NCORES)]
    for sid in order:
        cand = sorted(range(NCORES), key=lambda c: (loads[c], c))
        for c in cand:
            if len(members[c]) < SEG_PER_CORE:
                members[c].append(int(sid))
                loads[c] += int(chunks[sid])
                break
    # hill-climb swaps to shave the max load toward ceil(total/8)
    loads = [int(l) for l in loads]
    improved = True
    while improved:
        improved = False
        mx = max(loads)
        for h in [c for c in range(NCORES) if loads[c] == mx]:
            best = None
            for c in range(NCORES):
                if c == h:
                    continue
                for i, si in enumerate(members[h]):
                    for j, sj in enumerate(members[c]):
                        d = int(chunks[sj]) - int(chunks[si])
                        nm = max(loads[h] + d, loads[c] - d)
                        if nm < mx and (best is None or nm < best[0]):
                            best = (nm, c, i, j)
            if best:
                _, c, i, j = best
                si, sj = members[h][i], members[c][j]
                members[h][i], members[c][j] = sj, si
                loads[h] += int(chunks[sj]) - int(chunks[si])
                loads[c] += int(chunks[si]) - int(chunks[sj])
                improved = True
                break
    for c in range(NCORES):
        members[c].sort()

    ncch = [sum(int(chunks[s]) for s in members[c]) for c in range(NCORES)]
    NCH = max(ncch)  # exact: remainder DMA buffer handles NCH % CPB
    return {
        "seqL": seqL, "starts": starts, "chunks": chunks,
        "members": members, "NCH": NCH,
    }


def _repack_core(x_chw, plan, core):
    """Per-core DRAM stream [C, NCH*HW*R] (chunk-major; each chunk stored
    hw-major [hw, frame]), plus the per-(chunk,slot) accumulator mask."""
    NCH = plan["NCH"]
    members = plan["members"][core]
    out = np.full((C, NCH, HW, R), PAD, dtype=np.float32)
    mask_row = np.full((NCH, SEG_PER_CORE), -BIG, dtype=np.float32)
    cpos = 0
    for j, sid in enumerate(members):
        L = int(plan["seqL"][sid]); s0 = int(plan["starts"][sid])
        k = int(plan["chunks"][sid])
        segp = np.full((C, k * R, HW), PAD, dtype=np.float32)
        segp[:, :L, :] = x_chw[:, s0:s0 + L, :]
        out[:, cpos:cpos + k] = segp.reshape(C, k, R, HW).transpose(0, 1, 3, 2)
        mask_row[cpos:cpos + k, j] = BIG
        cpos += k
    return out.reshape(C, NCH * HW * R), mask_row.reshape(-1)


def _build_program(NCH):
    import concourse.bass as bass
    import concourse.tile as tile
    from concourse import bacc, mybir

    F32 = mybir.dt.float32
    SH = SEG_PER_CORE * HW          # 704
    SN = SEG_PER_CORE * NPART       # 16
    NB = (NCH + CPB - 1) // CPB

    nc = bacc.Bacc("TRN2", target_bir_lowering=False, debug=False,
                   num_devices=NCORES)
    x = nc.dram_tensor("x", [C, NCH * HW * R], F32, kind="ExternalInput")
    mask_d = nc.dram_tensor("mask", [C, NCH * SEG_PER_CORE], F32,
                            kind="ExternalInput")
    wt_d = nc.dram_tensor("wt", [C, O], F32, kind="ExternalInput")
    pvec_d = nc.dram_tensor("pvec", [C, 2 * SH], F32, kind="ExternalInput")
    qvec_d = nc.dram_tensor("qvec", [C, 2 * SN], F32, kind="ExternalInput")
    out_d = nc.dram_tensor("out", [C, 2 * SN], F32, kind="ExternalOutput")

    with tile.TileContext(nc) as tc:
        with tc.tile_pool(name="xin", bufs=3) as xin, \
             tc.tile_pool(name="cmp", bufs=3) as cmp_, \
             tc.tile_pool(name="meta", bufs=1) as meta, \
             tc.tile_pool(name="work", bufs=1) as work, \
             tc.tile_pool(name="psum", bufs=1, space="PSUM") as psum:
            wt = meta.tile([C, O], F32, tag="wt")
            pvec = meta.tile([C, 2 * SH], F32, tag="pvec")
            qvec = meta.tile([C, 2 * SN], F32, tag="qvec")
            mask = meta.tile([C, NCH * SEG_PER_CORE], F32, tag="mask")
            acc = work.tile([C, SH], F32, tag="acc")
            nc.vector.memset(acc[:], -BIG)
            # mask is read inside the stream loop - load it up front on the
            # ACT HWDGE queue so it doesn't delay the x stream on sync's
            nc.scalar.dma_start(mask[:], mask_d[:])

            # warm the ACT Ln/Exp tables during streaming (their lazy
            # loads otherwise land in the serial tail)
            warm = work.tile([C, 1], F32, tag="warm")
            nc.vector.memset(warm[:], 1.0)
            nc.scalar.activation(warm[:], warm[:],
                                 mybir.ActivationFunctionType.Ln)
            nc.scalar.activation(warm[:], warm[:],
                                 mybir.ActivationFunctionType.Exp)

            # phase 1: stream buffers; per-chunk max-reduce on DVE; masked
            # accumulate into the 4 segment slots on GPSIMD
            for b in range(NB):
                b0 = b * CPB
                cpb = min(CPB, NCH - b0)
                t = xin.tile([C, CPB * HW * R], F32, tag="xin")
                nc.sync.dma_start(
                    t[:, :cpb * HW * R],
                    x[:, b0 * HW * R:(b0 + cpb) * HW * R])
                cm = cmp_.tile([C, CPB * HW], F32, tag="cm")
                nc.vector.reduce_max(
                    cm[:, :cpb * HW].rearrange("p (c h) -> p c h", c=cpb),
                    t[:, :cpb * HW * R].rearrange("p (c h r) -> p c h r",
                                                  c=cpb, h=HW, r=R),
                    axis=mybir.AxisListType.X)
                tmp = cmp_.tile([C, CPB * SEG_PER_CORE * HW], F32, tag="tmp")
                cm_b = cm[:, :cpb * HW] \
                    .rearrange("p (c h) -> p c h", c=cpb) \
                    .unsqueeze(2).broadcast_to([C, cpb, SEG_PER_CORE, HW])
                mk_b = mask[:, b0 * SEG_PER_CORE:(b0 + cpb) * SEG_PER_CORE] \
                    .rearrange("p (c s) -> p c s", c=cpb) \
                    .unsqueeze(3).broadcast_to([C, cpb, SEG_PER_CORE, HW])
                tv = tmp[:, :cpb * SEG_PER_CORE * HW] \
                    .rearrange("p (c s h) -> p c s h", c=cpb, s=SEG_PER_CORE)
                nc.vector.tensor_tensor(tv, cm_b, mk_b, mybir.AluOpType.min)
                for c in range(cpb):
                    nc.vector.tensor_tensor(
                        acc[:],
                        acc[:],
                        tmp[:, c * SEG_PER_CORE * HW:
                            (c + 1) * SEG_PER_CORE * HW],
                        mybir.AluOpType.max)

            # meta loads issued after the stream DMAs so they don't delay
            # the first x buffer on the HWDGE queues
            nc.sync.dma_start(wt[:], wt_d[:])
            nc.sync.dma_start(pvec[:], pvec_d[:])
            nc.sync.dma_start(qvec[:], qvec_d[:])

            # conv 1x1 (2 O-halves x 2 psum banks each) + GeM tail.
            # Both halves share each op so the ACT engine loads each
            # activation table once instead of reloading per switch.
            gtile = work.tile([C, 2 * SN], F32, tag="g")
            t1 = work.tile([C, 2 * SH], F32, tag="t1x")
            for half in range(2):
                for ns in range(2):
                    y = psum.tile([C, SH // 2], F32, tag=f"y{half}{ns}")
                    nc.tensor.matmul(
                        y[:],
                        wt[:, half * 128:(half + 1) * 128],
                        acc[:, ns * (SH // 2):(ns + 1) * (SH // 2)],
                        start=True, stop=True)
                    nc.vector.tensor_scalar_max(
                        t1[:, half * SH + ns * (SH // 2):
                           half * SH + (ns + 1) * (SH // 2)], y[:], EPS)
            u = work.tile([C, 2 * SH], F32, tag="ux")
            nc.scalar.activation(u[:], t1[:],
                                 mybir.ActivationFunctionType.Ln,
                                 scale=float(RESCALE))
            v = work.tile([C, 2 * SH], F32, tag="vx")
            nc.vector.tensor_mul(v[:], u[:], pvec[:])
            w2 = work.tile([C, 2 * SH], F32, tag="wx")
            nc.scalar.activation(w2[:], v[:],
                                 mybir.ActivationFunctionType.Exp)
            s = work.tile([C, 2 * SN], F32, tag="sx")
            nc.vector.reduce_sum(
                s[:].rearrange("p (k one) -> p k one", one=1),
                w2[:].rearrange("p (k m) -> p k m", m=WPP),
                axis=mybir.AxisListType.X)
            nc.vector.tensor_scalar_max(s[:], s[:], float(SMIN))
            r2 = work.tile([C, 2 * SN], F32, tag="rx")
            nc.scalar.activation(r2[:], s[:],
                                 mybir.ActivationFunctionType.Ln,
                                 scale=float(1.0 / WPP))
            q2 = work.tile([C, 2 * SN], F32, tag="qx")
            nc.vector.tensor_mul(q2[:], r2[:], qvec[:])
            nc.vector.tensor_scalar_sub(q2[:], q2[:],
                                        float(np.log(RESCALE)))
            nc.scalar.activation(gtile[:], q2[:],
                                 mybir.ActivationFunctionType.Exp)
            nc.sync.dma_start(out_d[:], gtile[:])
    nc.compile()
    return nc


def _run_device(nc, in_maps):
    from concourse.bass_utils import run_bass_kernel_spmd
    res = run_bass_kernel_spmd(nc, in_maps, list(range(NCORES)))
    return res.results


def _make_in_maps(x, plan, W, p):
    x_chw = np.ascontiguousarray(x[0]).reshape(C, S, HW)
    wt = np.ascontiguousarray(W.T).astype(np.float32)               # [C, O]
    prow = np.repeat(p.astype(np.float32), WPP)                     # [HW]
    pvec = np.ascontiguousarray(
        np.broadcast_to(np.tile(prow, 2 * SEG_PER_CORE)[None, :],
                        (C, 2 * SEG_PER_CORE * HW))).astype(np.float32)
    qrow = (1.0 / p.astype(np.float32))                             # [NPART]
    qvec = np.ascontiguousarray(
        np.broadcast_to(np.tile(qrow, 2 * SEG_PER_CORE)[None, :],
                        (C, 2 * SEG_PER_CORE * NPART))).astype(np.float32)
    in_maps = []
    for core in range(NCORES):
        xc, mask_row = _repack_core(x_chw, plan, core)
        mask = np.ascontiguousarray(
            np.broadcast_to(mask_row[None, :],
                            (C, mask_row.shape[0]))).astype(np.float32)
        in_maps.append({
            "x": xc, "mask": mask,
            "wt": wt, "pvec": pvec, "qvec": qvec,
        })
    return in_maps


def kernel(x, seqL, W, p):
    x = np.asarray(x, dtype=np.float32)
    W = np.asarray(W, dtype=np.float32)
    p = np.asarray(p, dtype=np.float32)
    plan = _plan(seqL)

    in_maps = _make_in_maps(x, plan, W, p)

    key = plan["NCH"]
    if key not in _prog_cache:
        _prog_cache[key] = _build_program(plan["NCH"])
    nc = _prog_cache[key]

    results = _run_device(nc, in_maps)

    SN = SEG_PER_CORE * NPART
    out = np.zeros((B, O, NPART), dtype=np.float32)
    for core in range(NCORES):
        g = results[core]["out"]  # [C, 2*SN]
        for j, sid in enumerate(plan["members"][core]):
            for half in range(2):
                blk = g[:, half * SN + j * NPART: half * SN + (j + 1) * NPART]
                out[sid, half * 128:(half + 1) * 128, :] = blk
    return out


# revision 7
# speedup vs baseline: 2.1449x; 1.7668x over previous
"""Trainium2 Bass kernel for ragged segment-max + 1x1 conv + GeM pooling.

Problem: x [1,128,4096,16,11] f32 packed frames; seqL [32] ragged lengths;
W [256,128] 1x1-conv weight; p [4] GeM powers.  out [32, 256, 4] f32.

Strategy: shard whole segments across 8 cores (4 per core, LPT+swap
balanced).  x is stored in DRAM as bf16 (host cast - halves the HBM
stream, the dominant cost; bf16 rounding is ~0.4% rel, well inside the
2e-2 gate and not amplified by max/conv/GeM).  Per core: stream
frame-major 16-frame chunks (segment-aligned via -1e30 padding) and
max-fold each chunk with a 4-round in-place tensor_tensor tree on DVE
(bf16 2x mode).  Chunk maxes fold straight into a 4-slot segment
accumulator with masked min/max ops: acc[s] = max(acc[s], min(cm,
mask[chunk,s])) where mask is +/-BIG host data, expanded over hw on the
idle ACT engine - program stays uniform across cores while the
chunk->segment mapping is data.  The accumulator IS the pooled tensor:
1x1 conv on the PE (bf16) and the GeM tail (clip, ln, *p, exp, mean,
^(1/p)) on ACT/DVE in f32 follow directly - no scan / transpose / DRAM
round-trip / indirect gather.
"""
import sys

import numpy as np

if "/opt/trn_rl_repo" not in sys.path:
    sys.path.insert(0, "/opt/trn_rl_repo")

import ml_dtypes

BF16 = ml_dtypes.bfloat16

# problem constants
B, S, C, O = 32, 4096, 128, 256
H, Wd = 16, 11
HW = H * Wd                  # 176
SPLIT = [4, 4, 4, 4]         # h split sizes
NPART = len(SPLIT)           # 4
WPP = HW // NPART            # 44 elems per GeM part
EPS = 1e-6
NCORES = 8
SEG_PER_CORE = B // NCORES   # 4

R = 16                       # frames per chunk (segment-alignment quantum)
CPB = 3                      # chunks per DMA buffer
BIG = 3.0e38
PAD = -1.0e30
RESCALE = 150.0              # GeM computed on t*RESCALE: ACT Ln table is only
                             # valid on ~[2^-64, 2^64], so keep (RESCALE*t)^p
                             # inside it for t in [EPS, ~50]
SMIN = 44.0 * 2.0 ** -60     # clamp sum(w2) so mean stays in the Ln window


_prog_cache = {}


def _plan(seqL):
    """Host planning: segment->core assignment + per-core chunk layout."""
    seqL = np.asarray(seqL).astype(np.int64).reshape(B)
    assert seqL.sum() == S and (seqL > 0).all()
    starts = np.concatenate([[0], np.cumsum(seqL)[:-1]])
    chunks = (seqL + R - 1) // R  # padded chunk count per segment

    # LPT: assign segments to cores balancing padded chunk totals, 4 per core
    order = np.argsort(-chunks, kind="stable")
    loads = [0] * NCORES
    members = [[] for _ in range(NCORES)]
    for sid in order:
        cand = sorted(range(NCORES), key=lambda c: (loads[c], c))
        for c in cand:
            if len(members[c]) < SEG_PER_CORE:
                members[c].append(int(sid))
                loads[c] += int(chunks[sid])
                break

    # hill-climb swaps to shave the max load toward ceil(total/8)
    improved = True
    while improved:
        improved = False
        mx = max(loads)
        for h in [c for c in range(NCORES) if loads[c] == mx]:
            best = None
            for c in range(NCORES):
                if c == h:
                    continue
                for i, si in enumerate(members[h]):
                    for j, sj in enumerate(members[c]):
                        d = int(chunks[sj]) - int(chunks[si])
                        nm = max(loads[h] + d, loads[c] - d)
                        if nm < mx and (best is None or nm < best[0]):
                            best = (nm, c, i, j)
            if best:
                _, c, i, j = best
                si, sj = members[h][i], members[c][j]
                members[h][i], members[c][j] = sj, si
                loads[h] += int(chunks[sj]) - int(chunks[si])
                loads[c] += int(chunks[si]) - int(chunks[sj])
                improved = True
                break
    for c in range(NCORES):
        members[c].sort()

    ncch = [sum(int(chunks[s]) for s in members[c]) for c in range(NCORES)]
    NCH = max(ncch)  # exact: remainder DMA buffer handles NCH % CPB
    return {
        "seqL": seqL, "starts": starts, "chunks": chunks,
        "members": members, "NCH": NCH,
    }


def _repack_core(x_chw, plan, core):
    """Per-core DRAM stream [C, NCH*R*HW] bf16 (chunk-major, frame-major
    within a chunk), plus the per-(chunk,slot) accumulator mask."""
    NCH = plan["NCH"]
    members = plan["members"][core]
    out = np.full((C, NCH, R, HW), PAD, dtype=BF16)
    mask_row = np.full((NCH, SEG_PER_CORE), -BIG, dtype=np.float32)
    cpos = 0
    for j, sid in enumerate(members):
        L = int(plan["seqL"][sid]); s0 = int(plan["starts"][sid])
        k = int(plan["chunks"][sid])
        seg = out[:, cpos:cpos + k].reshape(C, k * R, HW)
        seg[:, :L, :] = x_chw[:, s0:s0 + L, :]
        mask_row[cpos:cpos + k, j] = BIG
        cpos += k
    return out.reshape(C, NCH * R * HW), mask_row.reshape(-1)


def _build_program(NCH):
    import concourse.bass as bass
    import concourse.tile as tile
    from concourse import bacc, mybir

    F32 = mybir.dt.float32
    BF = mybir.dt.bfloat16
    SH = SEG_PER_CORE * HW          # 704
    SN = SEG_PER_CORE * NPART       # 16
    NB = (NCH + CPB - 1) // CPB
    SL = SEG_PER_CORE

    nc = bacc.Bacc("TRN2", target_bir_lowering=False, debug=False,
                   num_devices=NCORES)
    x = nc.dram_tensor("x", [C, NCH * R * HW], BF, kind="ExternalInput")
    mask_d = nc.dram_tensor("mask", [C, NCH * SL], BF, kind="ExternalInput")
    wt_d = nc.dram_tensor("wt", [C, O], BF, kind="ExternalInput")
    pvec_d = nc.dram_tensor("pvec", [C, 2 * SH], F32, kind="ExternalInput")
    qvec_d = nc.dram_tensor("qvec", [C, 2 * SN], F32, kind="ExternalInput")
    out_d = nc.dram_tensor("out", [C, 2 * SN], F32, kind="ExternalOutput")

    with tile.TileContext(nc) as tc:
        with tc.tile_pool(name="xin", bufs=3) as xin, \
             tc.tile_pool(name="cmp", bufs=3) as cmp_, \
             tc.tile_pool(name="meta", bufs=1) as meta, \
             tc.tile_pool(name="work", bufs=1) as work, \
             tc.tile_pool(name="psum", bufs=1, space="PSUM") as psum:
            wt = meta.tile([C, O], BF, tag="wt")
            pvec = meta.tile([C, 2 * SH], F32, tag="pvec")
            qvec = meta.tile([C, 2 * SN], F32, tag="qvec")
            mask = meta.tile([C, NCH * SL], BF, tag="mask")
            maskx = meta.tile([C, NCH * SL * HW], BF, tag="maskx")
            acc = work.tile([C, SH], BF, tag="acc")
            nc.vector.memset(acc[:], -BIG)
            # mask is read inside the stream loop - load it up front on the
            # ACT HWDGE queue so it doesn't delay the x stream on sync's
            nc.scalar.dma_start(mask[:], mask_d[:])

            # warm the ACT Ln/Exp tables during streaming (their lazy
            # loads otherwise land in the serial tail)
            warm = work.tile([C, 1], F32, tag="warm")
            nc.vector.memset(warm[:], 1.0)
            nc.scalar.activation(warm[:], warm[:],
                                 mybir.ActivationFunctionType.Ln)
            nc.scalar.activation(warm[:], warm[:],
                                 mybir.ActivationFunctionType.Exp)

            # phase 1: stream buffers; in-place bf16 max-tree chunk fold on
            # DVE; masked accumulate into the 4 segment slots.  The
            # (chunk,slot) mask is expanded over hw per buffer on the idle
            # ACT engine so the DVE min op reads dense bf16.
            for b in range(NB):
                b0 = b * CPB
                cpb = min(CPB, NCH - b0)
                mslice = slice(b0 * SL * HW, (b0 + cpb) * SL * HW)
                nc.scalar.copy(
                    maskx[:, mslice].rearrange("p (c s h) -> p c s h",
                                               c=cpb, s=SL),
                    mask[:, b0 * SL:(b0 + cpb) * SL]
                    .rearrange("p (c s) -> p c s", c=cpb)
                    .unsqueeze(3).broadcast_to([C, cpb, SL, HW]))
                t = xin.tile([C, CPB * R * HW], BF, tag="xin")
                nc.sync.dma_start(
                    t[:, :cpb * R * HW],
                    x[:, b0 * R * HW:(b0 + cpb) * R * HW])
                tv = t[:, :cpb * R * HW].rearrange("p (c r h) -> p c r h",
                                                   c=cpb, r=R)
                for k in (8, 4, 2, 1):
                    nc.vector.tensor_tensor(
                        tv[:, :, 0:k, :], tv[:, :, 0:k, :],
                        tv[:, :, k:2 * k, :], mybir.AluOpType.max)
                tmp = cmp_.tile([C, CPB * SL * HW], BF, tag="tmp")
                cm_b = tv[:, :, 0:1, :].rearrange("p c one h -> p (c one) h") \
                    .unsqueeze(2).broadcast_to([C, cpb, SL, HW])
                tvw = tmp[:, :cpb * SL * HW] \
                    .rearrange("p (c s h) -> p c s h", c=cpb, s=SL)
                nc.vector.tensor_tensor(
                    tvw, cm_b,
                    maskx[:, mslice].rearrange("p (c s h) -> p c s h",
                                               c=cpb, s=SL),
                    mybir.AluOpType.min)
                for c in range(cpb):
                    nc.vector.tensor_tensor(
                        acc[:], acc[:],
                        tmp[:, c * SL * HW:(c + 1) * SL * HW],
                        mybir.AluOpType.max)

            # meta loads issued after the stream DMAs so they don't delay
            # the first x buffer on the HWDGE queues
            nc.sync.dma_start(wt[:], wt_d[:])
            nc.sync.dma_start(pvec[:], pvec_d[:])
            nc.sync.dma_start(qvec[:], qvec_d[:])

            # conv 1x1 (2 O-halves x 2 psum banks each) + GeM tail in f32.
            # Both halves share each op so the ACT engine loads each
            # activation table once instead of reloading per switch.
            gtile = work.tile([C, 2 * SN], F32, tag="g")
            t1 = work.tile([C, 2 * SH], F32, tag="t1x")
            for half in range(2):
                for ns in range(2):
                    y = psum.tile([C, SH // 2], F32, tag=f"y{half}{ns}")
                    nc.tensor.matmul(
                        y[:],
                        wt[:, half * 128:(half + 1) * 128],
                        acc[:, ns * (SH // 2):(ns + 1) * (SH // 2)],
                        start=True, stop=True)
                    nc.vector.tensor_scalar_max(
                        t1[:, half * SH + ns * (SH // 2):
                           half * SH + (ns + 1) * (SH // 2)], y[:], EPS)
            u = work.tile([C, 2 * SH], F32, tag="ux")
            nc.scalar.activation(u[:], t1[:],
                                 mybir.ActivationFunctionType.Ln,
                                 scale=float(RESCALE))
            v = work.tile([C, 2 * SH], F32, tag="vx")
            nc.vector.tensor_mul(v[:], u[:], pvec[:])
            w2 = work.tile([C, 2 * SH], F32, tag="wx")
            nc.scalar.activation(w2[:], v[:],
                                 mybir.ActivationFunctionType.Exp)
            s = work.tile([C, 2 * SN], F32, tag="sx")
            nc.vector.reduce_sum(
                s[:].rearrange("p (k one) -> p k one", one=1),
                w2[:].rearrange("p (k m) -> p k m", m=WPP),
                axis=mybir.AxisListType.X)
            nc.vector.tensor_scalar_max(s[:], s[:], float(SMIN))
            r2 = work.tile([C, 2 * SN], F32, tag="rx")
            nc.scalar.activation(r2[:], s[:],
                                 mybir.ActivationFunctionType.Ln,
                                 scale=float(1.0 / WPP))
            q2 = work.tile([C, 2 * SN], F32, tag="qx")
            nc.vector.tensor_mul(q2[:], r2[:], qvec[:])
            nc.vector.tensor_scalar_sub(q2[:], q2[:],
                                        float(np.log(RESCALE)))
            nc.scalar.activation(gtile[:], q2[:],
                                 mybir.ActivationFunctionType.Exp)
            nc.sync.dma_start(out_d[:], gtile[:])
    nc.compile()
    return nc


def _run_device(nc, in_maps):
    from concourse.bass_utils import run_bass_kernel_spmd
    res = run_bass_kernel_spmd(nc, in_maps, list(range(NCORES)))
    return res.results


def _make_in_maps(x, plan, W, p):
    x_chw = np.ascontiguousarray(x[0]).reshape(C, S, HW)
    wt = np.ascontiguousarray(W.T).astype(BF16)                     # [C, O]
    prow = np.repeat(p.astype(np.float32), WPP)                     # [HW]
    pvec = np.ascontiguousarray(
        np.broadcast_to(np.tile(prow, 2 * SEG_PER_CORE)[None, :],
                        (C, 2 * SEG_PER_CORE * HW))).astype(np.float32)
    qrow = (1.0 / p.astype(np.float32))                             # [NPART]
    qvec = np.ascontiguousarray(
        np.broadcast_to(np.tile(qrow, 2 * SEG_PER_CORE)[None, :],
                        (C, 2 * SEG_PER_CORE * NPART))).astype(np.float32)
    in_maps = []
    for core in range(NCORES):
        xc, mask_row = _repack_core(x_chw, plan, core)
        mask = np.ascontiguousarray(
            np.broadcast_to(mask_row.astype(BF16)[None, :],
                            (C, mask_row.shape[0])))
        in_maps.append({
            "x": xc, "mask": mask,
            "wt": wt, "pvec": pvec, "qvec": qvec,
        })
    return in_maps


def kernel(x, seqL, W, p):
    x = np.asarray(x, dtype=np.float32)
    W = np.asarray(W, dtype=np.float32)
    p = np.asarray(p, dtype=np.float32)
    plan = _plan(seqL)

    in_maps = _make_in_maps(x, plan, W, p)

    key = plan["NCH"]
    if key not in _prog_cache:
        _prog_cache[key] = _build_program(plan["NCH"])
    nc = _prog_cache[key]

    results = _run_device(nc, in_maps)

    SN = SEG_PER_CORE * NPART
    out = np.zeros((B, O, NPART), dtype=np.float32)
    for core in range(NCORES):
        g = results[core]["out"]  # [C, 2*SN]
        for j, sid in enumerate(plan["members"][core]):
            for half in range(2):
                blk = g[:, half * SN + j * NPART: half * SN + (j + 1) * NPART]
                out[sid, half * 128:(half + 1) * 128, :] = blk
    return out
